# revision 28
# baseline (speedup 1.0000x reference)
"""BiLSTM tagger on 8 TRN2 NeuronCores.

Strategy (hardcoded for B=64,T=512,V=30000,E=128,H=256,TAGS=50):
  - Data-parallel: batch sharded 8 ways (8 sequences/core); weights replicated.
  - Per core: embedding gather (indirect DMA) -> PE transpose -> x^T in SBUF;
    input projections xg = W_ih_aug @ [x; 1-m; 1] precomputed for all t as big
    matmuls into DRAM scratch (fp16); recurrences (l1 fwd+bwd step-interleaved,
    then l2 fwd+bwd) as dynamic Tile loops, 64 steps per iteration; classifier.
  - Masking: the (1-m) feature adds +/-60 to the f/i gate pre-activations at
    masked steps, freezing c exactly. Backward-direction h is then exactly 0 at
    masked steps. Forward h at masked steps is garbage on-chip; the host
    reconstructs masked-position logits exactly as cls_Wf @ h2f(len-1) + b
    (bwd part is 0 there), so no on-chip repair is needed.
  - Gate layout: gates on partitions (8 chunks of 128 = [i0 i1 f0 f1 o0 o1 g0 g1]),
    batch on free dim -> cheap pointwise; Whh stationary [128h x 128gate] bf16
    tiles (FWL), h moving [128, 8]. xg folded into PSUM via an identity matmul
    (issues early, off the critical path) so activations read PSUM directly.
  - The two directions of a layer are interleaved step-by-step so their serial
    pointwise chains overlap on different engines.
"""
import sys

sys.path.insert(0, "/opt/trn_rl_repo")
import contextlib

import numpy as np
import ml_dtypes

import concourse.bass as bass
import concourse.bacc as bacc
import concourse.mybir as mybir
import concourse.tile as tile
from concourse.bass import ds
from concourse.bass_utils import run_bass_kernel_spmd
from concourse.masks import make_identity

B, T, V, E, H, TAGS = 64, 512, 30000, 128, 256, 50
NCORES = 8
Bc = B // NCORES          # 8 sequences per core
TB = T * Bc               # 4096 tokens per core
STEPS_PER_BODY = 64
NBODY = T // STEPS_PER_BODY  # 8

f32 = mybir.dt.float32
bf16 = mybir.dt.bfloat16
fp16 = mybir.dt.float16
i32 = mybir.dt.int32

UNITS = ("1f", "1b", "2f", "2b")
KCNT = {"1f": 1, "1b": 1, "2f": 4, "2b": 4}       # 128-row K chunks of x features
REV = {"1f": False, "1b": True, "2f": False, "2b": True}
PHASES = (("1f", "1b"), ("2f", "2b"))

# gate chunk order g0 g1 i0 i1 f0 f1 o0 o1 (torch row order is i f g o).
# g first: its PSUM chunks finish first in the matmul burst, so the tanh
# can run while the sigmoid chunks are still streaming (off critical path).
PERM = np.concatenate([np.arange(512, 768), np.arange(0, 256),
                       np.arange(256, 512), np.arange(768, 1024)])

_CACHE = {}


def _prep_unit_weights(Wih, Whh, bih, bhh):
    """Host-side weight marshalling for one LSTM direction."""
    din = Wih.shape[1]
    Wp = np.asarray(Wih)[PERM]          # [1024, din]
    Up = np.asarray(Whh)[PERM]          # [1024, 256]
    bp = (np.asarray(bih) + np.asarray(bhh))[PERM]  # [1024]
    M = 1024
    k_cnt = din // 128
    # x-part lhsT: [din, M] -> k-chunk-major cols [128, k_cnt*M]
    WT = Wp.T.astype(np.float32)        # [din, 1024]
    wx = np.concatenate([WT[k * 128:(k + 1) * 128, :] for k in range(k_cnt)],
                        axis=1).astype(ml_dtypes.bfloat16)  # [128, k_cnt*M]
    # aug lhsT rows: feature0 = (1-m), feature1 = 1
    wa = np.zeros((2, M), np.float32)
    wa[0, 256:512] = -60.0   # i rows: -60*(1-m)
    wa[0, 512:768] = 60.0    # f rows: +60*(1-m)
    wa[1, :] = bp
    wa = wa.astype(ml_dtypes.bfloat16)
    # Whh lhsT: [256, 1024] -> [128, 2*1024]
    UT = Up.T.astype(np.float32)
    wh = np.concatenate([UT[0:128, :], UT[128:256, :]], axis=1).astype(ml_dtypes.bfloat16)
    return wx, wa, wh


def _build_program():
    nc = bacc.Bacc("TRN2", target_bir_lowering=False, debug=False, num_devices=NCORES)
    emb_d = nc.dram_tensor("emb", [V, E], f32, kind="ExternalInput")
    words_d = nc.dram_tensor("words", [TB, 1], i32, kind="ExternalInput")
    aug_d = nc.dram_tensor("aug", [2, TB], bf16, kind="ExternalInput")
    wxd, wad, whd, xgd = {}, {}, {}, {}
    for u in UNITS:
        wxd[u] = nc.dram_tensor(f"w{u}x", [128, KCNT[u] * 8 * 128], bf16, kind="ExternalInput")
        wad[u] = nc.dram_tensor(f"w{u}a", [2, 1024], bf16, kind="ExternalInput")
        whd[u] = nc.dram_tensor(f"w{u}h", [128, 2048], bf16, kind="ExternalInput")
        xgd[u] = nc.dram_tensor(f"xg{u}", [128, 8, TB], fp16)
    clsx_d = nc.dram_tensor("clsx", [128, 4 * TAGS], bf16, kind="ExternalInput")
    clsb_d = nc.dram_tensor("clsb", [TAGS, 1], f32, kind="ExternalInput")
    logits_d = nc.dram_tensor("logits", [TAGS, TB], f32, kind="ExternalOutput")
    h2f_d = nc.dram_tensor("h2f", [128, T * 2 * Bc], bf16, kind="ExternalOutput")

    ctx = contextlib.ExitStack()
    with tile.TileContext(nc) as tc, ctx:
        pp = ctx.enter_context(tc.tile_pool(name="persist", bufs=1))
        xT = pp.tile([128, TB], bf16, tag="xT")
        aug_sb = pp.tile([2, TB], bf16, tag="aug")
        ident = pp.tile([128, 128], f32, tag="ident")
        identh = pp.tile([128, 128], fp16, tag="identh")
        wx_sb = {u: pp.tile([128, KCNT[u] * 8 * 128], bf16, tag=f"wx{u}", name=f"wx{u}") for u in UNITS}
        wa_sb = {u: pp.tile([2, 1024], bf16, tag=f"wa{u}", name=f"wa{u}") for u in UNITS}
        wh_sb = {u: pp.tile([128, 2048], bf16, tag=f"wh{u}", name=f"wh{u}") for u in UNITS}
        cls_sb = pp.tile([128, 4 * TAGS], bf16, tag="clsx")
        clsb_sb = pp.tile([TAGS, 1], f32, tag="clsb")
        hs = {u: pp.tile([128, T, 2, Bc], bf16, tag=f"hs{u}", name=f"hs{u}") for u in UNITS}
        hcar = {u: pp.tile([128, 2, Bc], bf16, tag=f"hc{u}", name=f"hc{u}") for u in UNITS}
        ccar = {u: pp.tile([128, 2, Bc], f32, tag=f"cc{u}", name=f"cc{u}") for u in UNITS}

        # ---- load weights / constants
        make_identity(nc, ident[:])
        nc.vector.tensor_copy(identh[:], ident[:])
        for u in UNITS:
            nc.sync.dma_start(wx_sb[u][:], wxd[u][:])
            nc.sync.dma_start(wa_sb[u][:], wad[u][:])
            nc.sync.dma_start(wh_sb[u][:], whd[u][:])
        nc.sync.dma_start(cls_sb[:], clsx_d[:])
        nc.sync.dma_start(clsb_sb[:], clsb_d[:])
        nc.sync.dma_start(aug_sb[:], aug_d[:])
        for u in UNITS:
            nc.vector.memset(hcar[u][:, :, :], 0.0)
            nc.vector.memset(ccar[u][:, :, :], 0.0)

        # ---- embedding gather + transpose into xT
        with tc.tile_pool(name="gat", bufs=3) as gp, \
             tc.tile_pool(name="gps", bufs=3, space="PSUM") as gps:
            for n in range(TB // 128):
                idx = gp.tile([128, 1], i32, tag="idx")
                nc.sync.dma_start(idx[:], words_d[n * 128:(n + 1) * 128, :])
                xt = gp.tile([128, 128], f32, tag="xt")
                nc.gpsimd.indirect_dma_start(
                    out=xt[:], out_offset=None, in_=emb_d[:, :],
                    in_offset=bass.IndirectOffsetOnAxis(ap=idx[:, :1], axis=0))
                pst = gps.tile([128, 128], f32, tag="pst")
                nc.tensor.transpose(out=pst[:], in_=xt[:], identity=ident[:])
                nc.vector.tensor_copy(xT[:, n * 128:(n + 1) * 128], pst[:])

        # ---- xg precompute
        def xg_precompute(u, rhs_of_k):
            k_cnt = KCNT[u]
            # Emit blocks in the order the recurrence consumes them, so the
            # blocks a phase needs first get scheduler priority in the tail.
            n_order = range(TB // 512 - 1, -1, -1) if REV[u] else range(TB // 512)
            with tc.tile_pool(name=f"xp{u}", bufs=2, space="PSUM") as xps, \
                 tc.tile_pool(name=f"xs{u}", bufs=4) as xsb:
                for n in n_order:
                    nsl = slice(n * 512, (n + 1) * 512)
                    for m in range(8):
                        psm = xps.tile([128, 512], f32, tag="ps")
                        for k in range(k_cnt):
                            nc.tensor.matmul(
                                out=psm[:],
                                lhsT=wx_sb[u][:, (k * 8 + m) * 128:(k * 8 + m + 1) * 128],
                                rhs=rhs_of_k(k, n),
                                start=(k == 0), stop=False)
                        nc.tensor.matmul(
                            out=psm[:],
                            lhsT=wa_sb[u][:, m * 128:(m + 1) * 128],
                            rhs=aug_sb[:, nsl],
                            start=False, stop=True)
                        # Keep staging copies off the scalar engine: it is the
                        # recurrence bottleneck and xg work overlaps the phases.
                        stg = xsb.tile([128, 512], fp16, tag="stg")
                        nc.vector.tensor_copy(stg[:], psm[:])
                        nc.sync.dma_start(xgd[u][:, m, nsl], stg[:])

        def l1_rhs(k, n):
            return xT[:, n * 512:(n + 1) * 512]

        xg_precompute("1f", l1_rhs)
        xg_precompute("1b", l1_rhs)

        # ---- recurrence phase: two directions interleaved step-by-step
        def phase(units):
            with tc.tile_pool(name=f"rc{units[0]}", bufs=2) as rp, \
                 tc.tile_pool(name=f"rps{units[0]}", bufs=2, space="PSUM") as rps, \
                 tc.tile_pool(name=f"rtmp{units[0]}", bufs=4) as tp:
                with tc.For_i(0, NBODY, hint_engines=(mybir.EngineType.PE,)) as i:
                    CB = STEPS_PER_BODY * Bc   # columns per body
                    xb, hstage, t0s = {}, {}, {}
                    for u in units:
                        if REV[u]:
                            col0 = i * (-CB) + (TB - CB)
                            t0s[u] = i * (-STEPS_PER_BODY) + (T - STEPS_PER_BODY)
                        else:
                            col0 = i * CB
                            t0s[u] = i * STEPS_PER_BODY
                        xb[u] = rp.tile([128, 8, CB], fp16, tag=f"xb{u}", name=f"xb{u}")
                        # Load in 2 chunks ordered by consumption (reversed
                        # direction eats the tail first) so the first steps
                        # of a body start before the whole tile lands.
                        QC = CB // 2
                        order = (1, 0) if REV[u] else (0, 1)
                        for q in order:
                            nc.sync.dma_start(xb[u][:, :, q * QC:(q + 1) * QC],
                                              xgd[u][:, :, ds(col0 + q * QC, QC)])
                        hstage[u] = rp.tile([128, STEPS_PER_BODY, 2, Bc], bf16,
                                            tag=f"hst{u}", name=f"hst{u}")
                    for us in range(STEPS_PER_BODY):
                        slot = {u: (STEPS_PER_BODY - 1 - us) if REV[u] else us
                                for u in units}
                        pss, sg = {}, {}
                        # PE: open psum with xg (identity matmul), add Whh @ h.
                        # The g gates of both units share one psum bank (tanh
                        # depends on few matmuls, ready early in the burst);
                        # each unit's sigmoid tile's 13 close out its burst.
                        # 6 banks total leaves 2 for the xg pool so l2's xg
                        # precompute can overlap the phases.
                        psgAB = rps.tile([128, 2, 2, Bc], f32, tag="ggAB",
                                         name="ggAB")
                        psg = {u: psgAB[:, ui, :, :] for ui, u in enumerate(units)}
                        for u in units:
                            sl = slot[u]
                            bc = sl * Bc
                            pss[u] = rps.tile([128, 6, Bc], f32, tag=f"gs{u}",
                                              name=f"gs{u}")
                            if us == 0:
                                hprev = hcar[u]
                            else:
                                psl = sl + 1 if REV[u] else sl - 1
                                hprev = hstage[u][:, psl, :, :]
                            nc.tensor.matmul(out=psg[u][:, :, :], lhsT=identh[:],
                                             rhs=xb[u][:, 0:2, bc:bc + Bc],
                                             start=True, stop=False)
                            for m in range(2):
                                for k in range(2):
                                    nc.tensor.matmul(
                                        out=psg[u][:, m, :],
                                        lhsT=wh_sb[u][:, (k * 8 + m) * 128:(k * 8 + m + 1) * 128],
                                        rhs=hprev[:, k, :],
                                        start=False, stop=(k == 1))
                            nc.tensor.matmul(out=pss[u][:, :, :], lhsT=identh[:],
                                             rhs=xb[u][:, 2:8, bc:bc + Bc],
                                             start=True, stop=False)
                            for m in range(2, 8):
                                for k in range(2):
                                    nc.tensor.matmul(
                                        out=pss[u][:, m - 2, :],
                                        lhsT=wh_sb[u][:, (k * 8 + m) * 128:(k * 8 + m + 1) * 128],
                                        rhs=hprev[:, k, :],
                                        start=False, stop=(k == 1))
                        # ACT: gate nonlinearities straight from PSUM.
                        for u in units:
                            sg[u] = tp.tile([128, 8, Bc], f32, tag=f"sg{u}",
                                            name=f"sg{u}")
                            nc.scalar.activation(sg[u][:, 0:2, :], psg[u][:, :, :],
                                                 mybir.ActivationFunctionType.Tanh)
                            nc.scalar.activation(sg[u][:, 2:8, :], pss[u][:, :, :],
                                                 mybir.ActivationFunctionType.Sigmoid)
                        # DVE: c update
                        for u in units:
                            csf = tp.tile([128, 2, Bc], f32, tag=f"csf{u}",
                                          name=f"csf{u}")
                            nc.vector.tensor_tensor(out=csf[:, :, :], in0=sg[u][:, 4:6, :],
                                                    in1=ccar[u][:, :, :],
                                                    op=mybir.AluOpType.mult)
                            t1 = tp.tile([128, 2, Bc], f32, tag=f"t1{u}",
                                         name=f"t1{u}")
                            nc.vector.tensor_tensor(out=t1[:, :, :], in0=sg[u][:, 2:4, :],
                                                    in1=sg[u][:, 0:2, :],
                                                    op=mybir.AluOpType.mult)
                            nc.vector.tensor_tensor(out=ccar[u][:, :, :], in0=csf[:, :, :],
                                                    in1=t1[:, :, :],
                                                    op=mybir.AluOpType.add)
                        # ACT tanh(c) then DVE h = sig(o)*tanh(c)
                        for u in units:
                            tc2 = tp.tile([128, 2, Bc], f32, tag=f"tc{u}",
                                          name=f"tc{u}")
                            nc.scalar.activation(tc2[:, :, :], ccar[u][:, :, :],
                                                 mybir.ActivationFunctionType.Tanh)
                            nc.vector.tensor_tensor(out=hstage[u][:, slot[u], :, :],
                                                    in0=sg[u][:, 6:8, :],
                                                    in1=tc2[:, :, :],
                                                    op=mybir.AluOpType.mult)
                        # Half-way flush: the finished half of hstage goes to
                        # hs on the (otherwise idle) GpSimd engine while the
                        # second half of the body computes.
                        if us == STEPS_PER_BODY // 2 - 1:
                            HF = STEPS_PER_BODY // 2
                            for u in units:
                                lo = HF if REV[u] else 0
                                nc.gpsimd.tensor_copy(
                                    hs[u][:, ds(t0s[u] + lo, HF), :, :],
                                    hstage[u][:, lo:lo + HF, :, :])
                    # flush the second half + carry (off critical path)
                    HF = STEPS_PER_BODY // 2
                    for u in units:
                        lo = 0 if REV[u] else HF
                        nc.gpsimd.tensor_copy(hs[u][:, ds(t0s[u] + lo, HF), :, :],
                                              hstage[u][:, lo:lo + HF, :, :])
                        last_slot = 0 if REV[u] else STEPS_PER_BODY - 1
                        nc.vector.tensor_copy(hcar[u][:, :, :],
                                              hstage[u][:, last_slot, :, :])

        phase(("1f", "1b"))

        def l2_rhs(k, n):
            src = hs["1f"] if k < 2 else hs["1b"]
            return src[:, n * 64:(n + 1) * 64, k % 2, :]

        xg_precompute("2f", l2_rhs)
        xg_precompute("2b", l2_rhs)

        phase(("2f", "2b"))

        # ---- classifier
        with tc.tile_pool(name="cl", bufs=3) as cp, \
             tc.tile_pool(name="cps", bufs=3, space="PSUM") as cps:
            for n in range(TB // 512):
                psm = cps.tile([TAGS, 512], f32, tag="ps")
                for k in range(4):
                    src = hs["2f"] if k < 2 else hs["2b"]
                    nc.tensor.matmul(
                        out=psm[:],
                        lhsT=cls_sb[:, k * TAGS:(k + 1) * TAGS],
                        rhs=src[:, n * 64:(n + 1) * 64, k % 2, :],
                        start=(k == 0), stop=(k == 3))
                lg = cp.tile([TAGS, 512], f32, tag="lg")
                nc.vector.tensor_scalar_add(lg[:], psm[:], clsb_sb[:, :1])
                nc.sync.dma_start(logits_d[:, n * 512:(n + 1) * 512], lg[:])
        nc.sync.dma_start(h2f_d[:, :], hs["2f"][:, :, :, :])

    nc.compile()
    return nc


def kernel(**inputs):
    words = np.asarray(inputs["words"]).astype(np.int32)      # [B, T]
    lengths = np.asarray(inputs["lengths"]).astype(np.int32)  # [B]
    emb = np.asarray(inputs["emb"], dtype=np.float32)

    if "nc" not in _CACHE:
        _CACHE["nc"] = _build_program()
    nc = _CACHE["nc"]

    mask = (lengths[:, None] > np.arange(T)[None, :]).astype(np.float32)  # [B,T]
    wprep = {u: _prep_unit_weights(inputs[f"l{u}_Wih"], inputs[f"l{u}_Whh"],
                                   inputs[f"l{u}_bih"], inputs[f"l{u}_bhh"])
             for u in UNITS}
    clsW = np.asarray(inputs["cls_W"], dtype=np.float32)      # [50, 512]
    CT = clsW.T  # [512, 50]
    clsx = np.concatenate([CT[k * 128:(k + 1) * 128, :] for k in range(4)],
                          axis=1).astype(ml_dtypes.bfloat16)  # [128, 200]
    clsb = np.asarray(inputs["cls_b"], dtype=np.float32).reshape(TAGS, 1)

    in_maps = []
    for c in range(NCORES):
        bsl = slice(c * Bc, (c + 1) * Bc)
        w_c = words[bsl]                      # [Bc, T]
        m_c = mask[bsl]                       # [Bc, T]
        words_tm = np.ascontiguousarray(w_c.T).reshape(TB, 1)
        aug = np.stack([(1.0 - m_c.T).reshape(TB), np.ones(TB, np.float32)]
                       ).astype(ml_dtypes.bfloat16)           # [2, TB]
        im = {"emb": emb, "words": words_tm, "aug": aug,
              "clsx": clsx, "clsb": clsb}
        for u in UNITS:
            wx, wa, wh = wprep[u]
            im[f"w{u}x"] = wx
            im[f"w{u}a"] = wa
            im[f"w{u}h"] = wh
        in_maps.append(im)

    _CACHE["in_maps"] = in_maps
    res = run_bass_kernel_spmd(nc, in_maps, list(range(NCORES)))
    out = np.empty((B, T, TAGS), np.float32)
    CWf = clsW[:, :256]                                       # [50, 256]
    cb = clsb.reshape(TAGS)
    for c in range(NCORES):
        lg = res.results[c]["logits"]          # [50, TB], col = t*Bc + b
        out[c * Bc:(c + 1) * Bc] = lg.reshape(TAGS, T, Bc).transpose(2, 1, 0)
        h2f = np.asarray(res.results[c]["h2f"]).reshape(128, T, 2, Bc)
        for b in range(Bc):
            L = int(lengths[c * Bc + b])
            if L >= T:
                continue
            if L == 0:
                vec = np.zeros(H, np.float32)
            else:
                vec = h2f[:, L - 1, :, b].astype(np.float32).T.reshape(H)
            out[c * Bc + b, L:, :] = (CWf @ vec + cb)[None, :]
    return out


def bench(inputs):
    """Run once with NTFF tracing; returns HW exec_time_ns (and stashes trace)."""
    kernel(**inputs)  # ensure program built/cached
    nc = _CACHE["nc"]
    in_maps = _CACHE["in_maps"]
    import tempfile
    tmpdir = tempfile.mkdtemp(prefix="bilstm_trace_")
    res = run_bass_kernel_spmd(nc, in_maps, list(range(NCORES)), trace=True,
                               tmpdir=tmpdir)
    _CACHE["trace_dir"] = tmpdir
    _CACHE["last_bench"] = res
    print("trace dir:", tmpdir)
    return res.exec_time_ns


if __name__ == "__main__":
    import reference
    inputs = {k: np.asarray(v) for k, v in reference.setup_inputs().items()}
    got = kernel(**inputs)
    print(got.shape, got.dtype)
